# revision 13
# baseline (speedup 1.0000x reference)
"""Multi-head attention (B=2, S=2048, D=1024, H=16, Dh=64) on 8 trn2 cores.

Sharding: data-parallel over batch (2) x tensor-parallel over heads (4 groups
of 4 heads). Each core computes, for its batch b and head group hg:
  qT/kT = (W x^T) in [j, s] layout, v in [t, d] layout (all bf16),
  scoresT[t, s] = kT^T q / 8 per head (row-packed K=64 matmul pairs),
  expST = exp(scoresT) on ACT (fp32 psum in, bf16 out),
  ctxT[d, s] + softmax denominator via a ones-column appended to v (M=65),
  out_partial[s, :] = ctxT^T Wo_shard^T + bo (fp32 out).
Host sums the 4 head-group partials per batch.

v2 schedule: one global 128-step stream over (chunk, tt) with the ctx
matmuls lagging their scores by one step (hides the exp latency, also
across chunk boundaries), PE warm-up matmuls during the initial DMA
wait, per-ki-sliced weight/input DMAs across 4 queues, softmax
reciprocal on the vector engine, and a finer-grained tail.
"""

import ml_dtypes
import numpy as np

import concourse.bacc as bacc
import concourse.mybir as mybir
import concourse.tile as tile
from concourse.bass_utils import run_bass_kernel_spmd

F32 = mybir.dt.float32
BF16 = mybir.dt.bfloat16
I16 = mybir.dt.int16

S = 2048
DM = 1024
JL = 256  # local projection width = 4 heads * 64
HL = 4
DH = 64
P = 128
NK = DM // P
NJT = JL // P
NSB = S // 512
NTT = S // P
SCALE = 1.0 / np.sqrt(DH)
# Schraudolph exp in bf16 bit space, computed on the DVE as one
# tensor_scalar: bits_i16 = int16(x * (128*log2e*SCALE) + (16256 - C)).
# C=5.5 centers the sawtooth (max rel err ~3.3%, HW-validated); the
# residual zero-mean noise washes out in the softmax ratio (~0.1% on ctx).
EXP_A = float(128.0 * np.log2(np.e) * SCALE)
EXP_B = 16256.0 - 5.5

_CACHE = {}

_GAT_ORIG = bacc.get_activation_tables


def _gat_pinned(arch):
    t = _GAT_ORIG(arch)
    keep = "natural_log_exp_and_others"
    E, L = mybir.ActivationFunctionType.Exp, mybir.ActivationFunctionType.Ln
    if keep in t and E in t[keep] and L in t[keep]:
        for name, funcs in t.items():
            if name != keep:
                funcs.discard(E)
                funcs.discard(L)
    return t


bacc.get_activation_tables = _gat_pinned


def _spread(closures, d0, d1):
    """Assign due-dates d0..d1 evenly across a list of closures."""
    n = len(closures)
    out = []
    for i, f in enumerate(closures):
        due = d0 if n <= 1 else d0 + (d1 - d0) * i // (n - 1)
        out.append((due, f))
    return out


def build_nc(zero_bias=False):
    nc = bacc.Bacc("TRN2", target_bir_lowering=False, debug=False, num_devices=8)

    QT = nc.declare_dram_parameter("QT", [DM, S], BF16, isOutput=False)
    KT = nc.declare_dram_parameter("KT", [DM, S], BF16, isOutput=False)
    VT = nc.declare_dram_parameter("VT", [DM, S], BF16, isOutput=False)
    WQT = nc.declare_dram_parameter("WQT", [DM, JL], BF16, isOutput=False)
    WKT = nc.declare_dram_parameter("WKT", [DM, JL], BF16, isOutput=False)
    WVT = nc.declare_dram_parameter("WVT", [DM, JL], BF16, isOutput=False)
    WOT = nc.declare_dram_parameter("WOT", [JL, DM], BF16, isOutput=False)
    BQ = nc.declare_dram_parameter("BQ", [JL], F32, isOutput=False)
    BK = nc.declare_dram_parameter("BK", [JL], F32, isOutput=False)
    BV = nc.declare_dram_parameter("BV", [JL], F32, isOutput=False)
    BO = nc.declare_dram_parameter("BO", [DM], F32, isOutput=False)
    # bf16 partials (summed in fp32 on the host) — halves the output DMA
    OUT = nc.declare_dram_parameter("OUT", [S, DM], BF16, isOutput=True)

    with tile.TileContext(nc) as tc:
        with (
            tc.tile_pool(name="singles", bufs=1) as singles,
            tc.tile_pool(name="spsum", bufs=2, space="PSUM") as spool,
            tc.tile_pool(name="cpsum", bufs=4, space="PSUM") as cpool,
            tc.tile_pool(name="xin", bufs=4) as xpool,
            tc.tile_pool(name="exps", bufs=6) as epool,
            tc.tile_pool(name="outs", bufs=4) as outpool,
        ):
            wqt = singles.tile([P, NK, JL], BF16)
            wkt = singles.tile([P, NK, JL], BF16)
            wvt = singles.tile([P, NK, JL], BF16)
            wot = singles.tile([P, NJT, DM], BF16)
            bq_sb = singles.tile([P, NJT], F32)
            bk_sb = singles.tile([P, NJT], F32)
            bvb = singles.tile([P, JL], F32)
            bob = singles.tile([P, DM], F32)
            qt_sb = singles.tile([P, NJT, S], BF16)
            kt_sb = singles.tile([P, NJT, S], BF16)
            vaug = singles.tile([P, NTT, HL, DH + 1], BF16)
            kxin = singles.tile([P, NSB, NK, 512], BF16)
            qxin0 = singles.tile([P, NK, 512], BF16)
            ctxT = singles.tile([P, NJT, S], BF16)
            # head h's denominator at partition h*32 (legal engine bases)
            den_sb = singles.tile([P, NSB, 512], F32)
            rec_sb = singles.tile([P, NSB, 512], BF16)
            lntmp = singles.tile([P, 512], F32)
            sel = [singles.tile([P, P], BF16, name=f"sel{jt}") for jt in range(NJT)]

            # ---- constant init (vector) -- first so PE warm-up can start
            for jt in range(NJT):
                nc.vector.memset(sel[jt], 0.0)
                for h2 in range(2):
                    r = (jt * 2 + h2) * 32
                    nc.vector.memset(
                        sel[jt][r : r + 1, h2 * DH : (h2 + 1) * DH], 1.0
                    )
            nc.vector.memset(den_sb, 1.0)
            nc.vector.memset(vaug[:, :, :, DH : DH + 1], 1.0)

            # ---- PE warm-up: a short burst during the initial DMA wait
            wu_n = [0]

            def wu_burst(n):
                for _ in range(n):
                    wu = spool.tile(
                        [P, 64], F32, tag="sps", name=f"wu{wu_n[0]}"
                    )
                    wu_n[0] += 1
                    nc.tensor.matmul(
                        wu, sel[0], sel[1][:, 0:64], start=True, stop=True
                    )

            # ---- DMA helpers. Trigger issue is expensive (~600ns each,
            # ~4 outstanding per queue), so use few, coarse transfers and
            # assign queues by priority.
            pro_q = [nc.sync, nc.scalar, nc.gpsimd]
            run_q = [nc.sync, nc.gpsimd]
            qi = [0]

            def dma(out, in_, qs=run_q):
                if not isinstance(qs, (list, tuple)):
                    qs = [qs]
                qs[qi[0] % len(qs)].dma_start(out=out, in_=in_)
                qi[0] += 1

            def dma_w(w_sb, W, qs=run_q):
                # whole weight tile in one transfer
                dma(w_sb, W.rearrange("(k p) j -> p k j", p=P), qs)

            def dma_x(dst, X, tb, qs=run_q, parts=2):
                # [P, NK, 512] input block in `parts` transfers
                kstep = NK // parts
                for a in range(parts):
                    dma(
                        dst[:, a * kstep : (a + 1) * kstep, :],
                        X[
                            a * kstep * P : (a + 1) * kstep * P,
                            tb * 512 : (tb + 1) * 512,
                        ].rearrange("(k p) s -> p k s", p=P),
                        qs,
                    )

            # Lazy xpool inputs: DMA trigger closure allocates the tile; the
            # consuming matmul closures read it via the shared slot.
            def lazy_x(X, tb, name, qs=run_q):
                slot = {}

                def trig():
                    x = xpool.tile([P, NK, 512], BF16, tag="xin", name=name)
                    slot["x"] = x
                    dma_x(x, X, tb, qs)

                return slot, trig

            # ---- projection closure factories (per-ki granularity)
            def proj_kq_mms(dst, w_sb, b_sb, xin, tb, jt, pfx):
                st = {}

                def mk(ki):
                    def emit():
                        x = xin["x"] if isinstance(xin, dict) else xin
                        if ki == 0:
                            st["p"] = cpool.tile(
                                [P, 512], F32, tag="cb", name=f"pp{pfx}{tb}_{jt}"
                            )
                        nc.tensor.matmul(
                            st["p"],
                            w_sb[:, ki, jt * P : (jt + 1) * P],
                            x[:, ki, :],
                            start=(ki == 0),
                            stop=(ki == NK - 1),
                        )
                        if ki == NK - 1:
                            dsl = dst[:, jt, tb * 512 : (tb + 1) * 512]
                            if zero_bias:
                                nc.vector.tensor_copy(dsl, st["p"])
                            else:
                                nc.vector.tensor_scalar_add(
                                    dsl, st["p"], b_sb[:, jt : jt + 1]
                                )

                    return emit

                return [mk(ki) for ki in range(NK)]

            def proj_v_mms(tb, xin):
                st = {}
                out = []

                def mk(tl, ki):
                    def emit():
                        x = xin["x"] if isinstance(xin, dict) else xin
                        if tl % 2 == 0 and ki == 0:
                            st[tl // 2] = cpool.tile(
                                [P, 512], F32, tag="cb", name=f"pv{tb}_{tl // 2}"
                            )
                        nc.tensor.matmul(
                            st[tl // 2][:, (tl % 2) * JL : (tl % 2 + 1) * JL],
                            x[:, ki, tl * P : (tl + 1) * P],
                            wvt[:, ki, :],
                            start=(ki == 0),
                            stop=(ki == NK - 1),
                        )
                        if ki == NK - 1:
                            tt = tb * 4 + tl
                            pvv = st[tl // 2][
                                :, (tl % 2) * JL : (tl % 2 + 1) * JL
                            ].rearrange("p (h d) -> p h d", h=HL)
                            if zero_bias:
                                nc.vector.tensor_copy(vaug[:, tt, :, 0:DH], pvv)
                            else:
                                nc.vector.tensor_add(
                                    vaug[:, tt, :, 0:DH],
                                    pvv,
                                    bvb.rearrange("p (h d) -> p h d", h=HL),
                                )

                    return emit

                for tl in range(4):
                    for ki in range(NK):
                        out.append(mk(tl, ki))
                return out

            # ---- softmax normalization (DVE reciprocal, PE broadcast).
            # reciprocal_approx_fast must run on all 128 partitions — custom
            # DVE ops mis-write with partition-offset APs. Unwritten den rows
            # are memset 1.0 and harmless; the tail recomputes idempotently.
            def recip_mk(sb, pr=None):
                def recip():
                    nc.vector.reciprocal_approx_fast(
                        out=lntmp[:, :], in_=den_sb[:, sb, :]
                    )
                    nc.vector.tensor_copy(rec_sb[:, sb, :], lntmp[:, :])

                return recip

            def scale_mk(sb, jt2):
                pr = slice(jt2 * DH, (jt2 + 1) * DH)

                def scale():
                    rb = cpool.tile([P, 512], F32, tag="cb", name=f"rb{sb}_{jt2}")
                    nc.tensor.matmul(
                        rb,
                        sel[jt2][pr, :],
                        rec_sb[pr, sb, :],
                        start=True,
                        stop=True,
                    )
                    seg = ctxT[:, jt2, sb * 512 : (sb + 1) * 512]
                    nc.vector.tensor_mul(seg, seg, rb)

                return scale

            def norm_pair_fillers(sb, base):
                return [
                    (base, recip_mk(sb)),
                    (base + 1, scale_mk(sb, 0)),
                    (base + 2, scale_mk(sb, 1)),
                ]

            def norm_one_fillers(sb, jt2, base):
                return [
                    (base, recip_mk(sb)),
                    (base + 2, scale_mk(sb, jt2)),
                ]

            # ---- output projection for one 128-row block st, emitted as
            # two half-blocks (nb) so the PE burst per step stays small and
            # ACT is never starved by an atomic 4-matmul group
            def outproj_nb(st_idx, nb, ots, tail=False):
                def emit():
                    po = cpool.tile(
                        [P, 512], F32, tag="cb", name=f"po{st_idx}_{nb}"
                    )
                    for jt2 in range(NJT):
                        nc.tensor.matmul(
                            po,
                            ctxT[:, jt2, st_idx * P : (st_idx + 1) * P],
                            wot[:, jt2, nb * 512 : (nb + 1) * 512],
                            start=(jt2 == 0),
                            stop=(jt2 == NJT - 1),
                        )
                    if nb == 0:
                        ots["t"] = outpool.tile(
                            [P, DM], BF16, tag="ot", name=f"ot{st_idx}"
                        )
                    osl = ots["t"][:, nb * 512 : (nb + 1) * 512]
                    if zero_bias:
                        if tail and nb == 1:
                            nc.scalar.activation(
                                out=osl,
                                in_=po,
                                func=mybir.ActivationFunctionType.Copy,
                            )
                        else:
                            nc.vector.tensor_copy(osl, po)
                    else:
                        nc.vector.tensor_add(
                            osl, po, bob[:, nb * 512 : (nb + 1) * 512]
                        )
                    # one DMA per 128-row block: fewer transfers shrink the
                    # end-of-kernel completion-sync teardown
                    if nb == 1:
                        if tail:
                            eng = nc.sync if st_idx % 2 == 0 else nc.scalar
                        else:
                            eng = nc.sync if st_idx % 2 == 0 else nc.gpsimd
                        eng.dma_start(
                            out=OUT[st_idx * P : (st_idx + 1) * P, :],
                            in_=ots["t"][:, :],
                        )

                return emit

            def outproj_fillers(st_idx, due, tail=False):
                ots = {}
                return [
                    (due, outproj_nb(st_idx, 0, ots, tail)),
                    (due + 1, outproj_nb(st_idx, 1, ots, tail)),
                ]

            def outproj_one(st_idx, tail=False):
                def emit():
                    for _, fn in outproj_fillers(st_idx, 0, tail):
                        fn()

                return emit

            # ---- prologue DMAs: minimize bytes before the first scores.
            # Critical = K-jt0 weights+inputs then Q-jt0, striped across all
            # four trigger queues (sync/scalar HW-DGE + vector/gpsimd SW-DGE).
            vx0 = xpool.tile([P, NK, 512], BF16, tag="xin", name="vx0")
            q4 = [nc.sync, nc.scalar, nc.gpsimd]

            def dma_w_jt(w_sb, W, jt, q):
                dma(
                    w_sb[:, :, jt * P : (jt + 1) * P],
                    W[:, jt * P : (jt + 1) * P].rearrange(
                        "(k p) j -> p k j", p=P
                    ),
                    q,
                )

            dma_w_jt(wkt, WKT, 0, nc.sync)
            dma_w_jt(wqt, WQT, 0, nc.scalar)
            dma_x(kxin[:, 0], KT, 0, q4, parts=4)
            dma_x(qxin0, QT, 0, q4, parts=4)
            dma_w_jt(wkt, WKT, 1, nc.gpsimd)
            dma_w_jt(wqt, WQT, 1, nc.gpsimd)
            if not zero_bias:
                dma(bk_sb, BK.rearrange("(n p) -> p n", p=P), nc.scalar)
                dma(bq_sb, BQ.rearrange("(n p) -> p n", p=P), nc.scalar)
            dma_x(kxin[:, 1], KT, 1, [nc.sync, nc.scalar])
            dma_w(wvt, WVT, nc.gpsimd)
            dma_x(vx0, VT, 0, [nc.sync, nc.scalar])
            if not zero_bias:
                dma(
                    bvb,
                    BV.reshape([1, JL])[:].to_broadcast((P, JL)),
                    nc.gpsimd,
                )

            # ---- prologue projections: only what scores(c0, tt0) needs;
            # everything else becomes loop filler. Warm-ups bridge the
            # initial DMA wait and start the PE clock ramp.
            wu_burst(16)
            for f in proj_kq_mms(kt_sb, wkt, bk_sb, kxin[:, 0], 0, 0, "k"):
                f()
            for f in proj_kq_mms(qt_sb, wqt, bq_sb, qxin0, 0, 0, "q"):
                f()

            # ---- per-chunk filler lists (due dates in local tt units)
            chunks = [(sb, jt) for sb in range(NSB) for jt in range(NJT)]
            fillers = [[] for _ in chunks]

            vx1, vx1_trig = lazy_x(VT, 1, "vx1", [nc.sync, nc.gpsimd])
            vx2, vx2_trig = lazy_x(VT, 2, "vx2", [nc.gpsimd, nc.sync])
            vx3, vx3_trig = lazy_x(VT, 3, "vx3", [nc.sync, nc.gpsimd])
            qx1, qx1_trig = lazy_x(QT, 1, "qx1", nc.sync)
            qx2, qx2_trig = lazy_x(QT, 2, "qx2", run_q)
            qx3, qx3_trig = lazy_x(QT, 3, "qx3", run_q)

            def wot_trig():
                for n in range(NJT):
                    dma(wot[:, n, :], WOT[n * P : (n + 1) * P, :])
                if not zero_bias:
                    dma(bob, BO.reshape([1, DM])[:].to_broadcast((P, DM)))

            # C0 = (sb0, jt0): all V projections, K-jt0 tb1-3 (needed at
            # tt 4/8/12), then the displaced prologue work K-jt1-tb0 and
            # Q-jt1-sb0 (needed at c1), plus input DMA triggers
            f = []
            f.append((0, vx1_trig))
            f.append((1, lambda: dma_x(kxin[:, 2], KT, 2, [nc.scalar, nc.gpsimd])))
            f.append((1, vx2_trig))
            f.append((2, lambda: dma_x(kxin[:, 3], KT, 3, [nc.scalar, nc.sync])))
            f.append((3, vx3_trig))
            f += _spread(proj_v_mms(0, vx0), 0, 2)
            f += _spread(
                proj_kq_mms(kt_sb, wkt, bk_sb, kxin[:, 1], 1, 0, "k"), 1, 3
            )
            f += _spread(proj_v_mms(1, vx1), 3, 6)
            f += _spread(
                proj_kq_mms(kt_sb, wkt, bk_sb, kxin[:, 2], 2, 0, "k"), 5, 7
            )
            f += _spread(proj_v_mms(2, vx2), 7, 9)
            f.append((8, wot_trig))
            f += _spread(
                proj_kq_mms(kt_sb, wkt, bk_sb, kxin[:, 3], 3, 0, "k"), 9, 11
            )
            f += _spread(proj_v_mms(3, vx3), 10, 13)
            f += _spread(proj_kq_mms(qt_sb, wqt, bq_sb, qxin0, 0, 1, "qb"), 10, 13)
            f += _spread(
                proj_kq_mms(kt_sb, wkt, bk_sb, kxin[:, 0], 0, 1, "kb"), 12, 14
            )
            fillers[0] = f

            # C1 = (sb0, jt1): K-jt1 tb1-3, Q sb1 (both jt)
            f = []
            f.append((0, qx1_trig))
            f += _spread(
                proj_kq_mms(kt_sb, wkt, bk_sb, kxin[:, 1], 1, 1, "kb"), 0, 3
            )
            f += _spread(
                proj_kq_mms(kt_sb, wkt, bk_sb, kxin[:, 2], 2, 1, "kb"), 4, 7
            )
            f += _spread(proj_kq_mms(qt_sb, wqt, bq_sb, qx1, 1, 0, "q"), 6, 9)
            f += _spread(
                proj_kq_mms(kt_sb, wkt, bk_sb, kxin[:, 3], 3, 1, "kb"), 8, 11
            )
            f += _spread(proj_kq_mms(qt_sb, wqt, bq_sb, qx1, 1, 1, "q"), 11, 14)
            fillers[1] = f

            # C2: norm(0) batched + outproj(0)
            f = []
            f += norm_pair_fillers(0, 0)
            for i, st_i in enumerate(range(0, 4)):
                f += outproj_fillers(st_i, 3 + 3 * i)
            fillers[2] = f

            # C3: Q sb2
            f = []
            f.append((0, qx2_trig))
            f += _spread(proj_kq_mms(qt_sb, wqt, bq_sb, qx2, 2, 0, "q"), 3, 6)
            f += _spread(proj_kq_mms(qt_sb, wqt, bq_sb, qx2, 2, 1, "q"), 8, 11)
            fillers[3] = f

            # C4: norm(1) batched + outproj(1)
            f = []
            f += norm_pair_fillers(1, 0)
            for i, st_i in enumerate(range(4, 8)):
                f += outproj_fillers(st_i, 3 + 3 * i)
            fillers[4] = f

            # C5: Q sb3
            f = []
            f.append((0, qx3_trig))
            f += _spread(proj_kq_mms(qt_sb, wqt, bq_sb, qx3, 3, 0, "q"), 3, 6)
            f += _spread(proj_kq_mms(qt_sb, wqt, bq_sb, qx3, 3, 1, "q"), 8, 11)
            fillers[5] = f

            # C6: norm(2) batched + outproj(2)
            f = []
            f += norm_pair_fillers(2, 0)
            for i, st_i in enumerate(range(8, 11)):
                f += outproj_fillers(st_i, 3 + 4 * i)
            fillers[6] = f

            # C7: norm(3,0) + outproj(2) st11
            f = []
            f += norm_one_fillers(3, 0, 2)
            f += outproj_fillers(11, 6)
            fillers[7] = f

            for fl in fillers:
                fl.sort(key=lambda e: e[0])

            # ---- main stream: scores lead, ctx lags one step
            def emit_scores(c, tt, use_dve=False):
                sb, jt = chunks[c]
                sps = spool.tile([P, 1024], F32, tag="sps", name=f"sps{c}_{tt}")
                for h2 in range(2):
                    ho = h2 * DH
                    nc.tensor.matmul(
                        sps[:, h2 * 512 : (h2 + 1) * 512],
                        kt_sb[ho : ho + DH, jt, tt * P : (tt + 1) * P],
                        qt_sb[ho : ho + DH, jt, sb * 512 : (sb + 1) * 512],
                        start=True,
                        stop=True,
                    )
                ex = epool.tile([P, 1024], BF16, tag="ex", name=f"ex{c}_{tt}")
                if use_dve:
                    # offload exp to the vector engine (ACT is the pacing
                    # resource): Schraudolph bits written via an int16 view
                    nc.vector.tensor_scalar(
                        ex.bitcast(I16),
                        sps,
                        EXP_A,
                        EXP_B,
                        mybir.AluOpType.mult,
                        mybir.AluOpType.add,
                    )
                else:
                    nc.scalar.activation(
                        out=ex,
                        in_=sps,
                        func=mybir.ActivationFunctionType.Exp,
                        scale=float(SCALE),
                    )
                return ex

            cstate = {}

            def emit_ctx(c, tt, ex):
                sb, jt = chunks[c]
                if tt == 0:
                    cstate[c] = [
                        cpool.tile(
                            [DH + 1, 512], F32, tag="cb", name=f"cps{c}_{i}"
                        )
                        for i in range(2)
                    ]
                cps = cstate[c]
                for h2 in range(2):
                    h = jt * 2 + h2
                    nc.tensor.matmul(
                        cps[h2],
                        vaug[:, tt, h, :],
                        ex[:, h2 * 512 : (h2 + 1) * 512],
                        start=(tt == 0),
                        stop=(tt == NTT - 1),
                    )
                if tt == NTT - 1:
                    for h2 in range(2):
                        h = jt * 2 + h2
                        nc.vector.tensor_copy(
                            den_sb[h * 32 : h * 32 + 1, sb, :],
                            cps[h2][DH : DH + 1, :],
                        )
                    for h2 in range(2):
                        dst = ctxT[
                            h2 * DH : (h2 + 1) * DH,
                            jt,
                            sb * 512 : (sb + 1) * 512,
                        ]
                        if c == len(chunks) - 1:
                            # ACT is idle after the final exp; freeing DVE
                            # lets the tail reciprocal chain start at once
                            nc.scalar.activation(
                                out=dst,
                                in_=cps[h2][0:DH, :],
                                func=mybir.ActivationFunctionType.Copy,
                            )
                        else:
                            nc.vector.tensor_copy(dst, cps[h2][0:DH, :])

            pend = None
            for g in range(len(chunks) * NTT):
                c, tt = divmod(g, NTT)
                ex = emit_scores(c, tt, use_dve=(g % 4 == 1))
                if pend is not None:
                    emit_ctx(*pend)
                pend = (c, tt, ex)
                fl = fillers[c]
                while fl and fl[0][0] <= tt:
                    fl.pop(0)[1]()
                if tt == NTT - 1:
                    while fl:
                        fl.pop(0)[1]()
            emit_ctx(*pend)

            # ---- tail: norm(3,1) then outproj(3)
            for _, fn in norm_one_fillers(3, 1, 0):
                fn()
            for st_i in range(12, 16):
                outproj_one(st_i, tail=True)()

    nc.compile()
    return nc


def _shard_inputs(Q, K, V, Wq, bq, Wk, bk, Wv, bv, Wo, bo):
    in_maps = []
    xt = {}
    for b in range(2):
        xt[b] = tuple(
            np.ascontiguousarray(np.asarray(a[b], dtype=np.float32).T).astype(
                ml_dtypes.bfloat16
            )
            for a in (Q, K, V)
        )
    for c in range(8):
        b, hg = c // 4, c % 4
        sl = slice(hg * JL, (hg + 1) * JL)
        qt, kt, vt = xt[b]
        in_maps.append(
            {
                "QT": qt,
                "KT": kt,
                "VT": vt,
                "WQT": np.ascontiguousarray(
                    np.asarray(Wq, np.float32)[sl, :].T
                ).astype(ml_dtypes.bfloat16),
                "WKT": np.ascontiguousarray(
                    np.asarray(Wk, np.float32)[sl, :].T
                ).astype(ml_dtypes.bfloat16),
                "WVT": np.ascontiguousarray(
                    np.asarray(Wv, np.float32)[sl, :].T
                ).astype(ml_dtypes.bfloat16),
                "WOT": np.ascontiguousarray(
                    np.asarray(Wo, np.float32)[:, sl].T
                ).astype(ml_dtypes.bfloat16),
                "BQ": np.ascontiguousarray(np.asarray(bq, np.float32)[sl]),
                "BK": np.ascontiguousarray(np.asarray(bk, np.float32)[sl]),
                "BV": np.ascontiguousarray(np.asarray(bv, np.float32)[sl]),
                "BO": (
                    np.ascontiguousarray(np.asarray(bo, np.float32))
                    if hg == 0
                    else np.zeros(DM, np.float32)
                ),
            }
        )
    return in_maps


def kernel(Q, K, V, Wq, bq, Wk, bk, Wv, bv, Wo, bo):
    zb = all(
        not np.any(np.asarray(b, np.float32)) for b in (bq, bk, bv, bo)
    )
    key = ("nc", zb)
    if key not in _CACHE:
        _CACHE[key] = build_nc(zero_bias=zb)
    nc = _CACHE[key]
    in_maps = _shard_inputs(Q, K, V, Wq, bq, Wk, bk, Wv, bv, Wo, bo)
    res = run_bass_kernel_spmd(nc, in_maps, list(range(8)))
    out = np.zeros((2, S, DM), np.float32)
    for c in range(8):
        out[c // 4] += res.results[c]["OUT"]
    return out

